# revision 5
# baseline (speedup 1.0000x reference)
"""Trainium2 Bass kernel for the GP linearized-Laplace (GPLLA) problem.

Math (reference):
    mean = X @ W_net + b_net                                   (n, a)
    Jz   = einsum('nd,das->nas', X, Jproj) * prior_std         (n, a, s)
    Kzz  = einsum('nas,nbs->abn', Jz, Jz)
    Kzx  = einsum('nas,mbs->abnm', Jz, Jx)
    tmp  = einsum('abnm,bcml->acnl', Kzx, inv)
    K2   = einsum('acnl,dcnl->adn', tmp, Kzx)
    out  = (mean, transpose(Kzz - K2, (2,0,1)))                (n,a), (n,a,d)

Sharding: data-parallel over the test-batch axis n (32 points per core on
8 cores); Jx and inv are replicated (streamed from HBM in fp8 with power-
of-two scaling to keep DMA under the per-core HBM roofline).

Per-core layout: rows are the joint index p = a*32 + n_loc (128 = full
partition dim); the big contractions run over s (resp. (b,m)) = 4096 as
32 K-chunks of 128 with fp8 moving operands.  The per-test-point diagonal
of the quadratic form is extracted with a block-diagonal mask multiply +
segmented reduce.
"""

import sys

for p in ("/opt/trn_rl_repo", "/root/.axon_site/_ro/trn_rl_repo"):
    if p not in sys.path:
        sys.path.insert(0, p)

from contextlib import ExitStack

import ml_dtypes
import numpy as np

import concourse.bass as bass
import concourse.mybir as mybir
import concourse.tile as tile
from concourse import bacc
from concourse.bass_utils import run_bass_kernel_spmd

N_TEST, N_TRAIN, A, D, S = 256, 1024, 4, 256, 4096
NCORES = 8
NL = N_TEST // NCORES          # 32 test points per core
MB = A * N_TRAIN               # 4096, joint (b, m) index
JX_SCALE = 16.0                # fp8 range scaling for Jx (exact power of 2)
INV_SCALE = 1024.0             # fp8 range scaling for inv

F32 = mybir.dt.float32
BF16 = mybir.dt.bfloat16
F8 = mybir.dt.float8e4

_PROG = None


def _build_program():
    nc = bacc.Bacc(
        "TRN2", target_bir_lowering=False, debug=False, num_devices=NCORES
    )

    xt_bf = nc.dram_tensor("xt_bf", [2, 128, NL], BF16, kind="ExternalInput")
    xt_f32 = nc.dram_tensor("xt_f32", [2, 128, NL], F32, kind="ExternalInput")
    wnet = nc.dram_tensor("wnet", [2, 128, A], F32, kind="ExternalInput")
    bb = nc.dram_tensor("bb", [NL, A], F32, kind="ExternalInput")
    jproj = nc.dram_tensor("jproj", [2, 128, A, S], BF16, kind="ExternalInput")
    jxt = nc.dram_tensor("jxt", [S, MB], F8, kind="ExternalInput")
    invr = nc.dram_tensor("invr", [MB, MB], F8, kind="ExternalInput")
    mask = nc.dram_tensor("mask", [128, 128], F32, kind="ExternalInput")

    mean_o = nc.dram_tensor("mean_o", [NL, A], F32, kind="ExternalOutput")
    klla_o = nc.dram_tensor("klla_o", [NL, A, A], F32, kind="ExternalOutput")

    Copy = mybir.ActivationFunctionType.Copy

    with tile.TileContext(nc) as tc, ExitStack() as ctx:
        const = ctx.enter_context(tc.tile_pool(name="const", bufs=1))
        small = ctx.enter_context(tc.tile_pool(name="small", bufs=1))
        big = ctx.enter_context(tc.tile_pool(name="big", bufs=1))
        jp_pool = ctx.enter_context(tc.tile_pool(name="jp", bufs=3))
        jx_pool = ctx.enter_context(tc.tile_pool(name="jx", bufs=6))
        inv_pool = ctx.enter_context(tc.tile_pool(name="inv", bufs=6))
        psum = ctx.enter_context(
            tc.tile_pool(name="psum", bufs=1, space=bass.MemorySpace.PSUM)
        )

        # ---- constants ----
        xtb = const.tile([128, 2, NL], BF16)
        nc.sync.dma_start(xtb[:], xt_bf[:].rearrange("c p n -> p c n"))
        xtf = const.tile([128, 2, NL], F32)
        nc.sync.dma_start(xtf[:], xt_f32[:].rearrange("c p n -> p c n"))
        wn = const.tile([128, 2, A], F32)
        nc.sync.dma_start(wn[:], wnet[:].rearrange("c p n -> p c n"))
        bbt = const.tile([NL, A], F32)
        nc.sync.dma_start(bbt[:], bb[:])
        msk = const.tile([128, 128], F32)
        nc.sync.dma_start(msk[:], mask[:])

        # ---- mean = X @ W_net + b ----
        ps_mean = psum.tile([NL, A], F32, tag="ps")
        for dc in range(2):
            nc.tensor.matmul(
                ps_mean[:],
                lhsT=xtf[:, dc, :],
                rhs=wn[:, dc, :],
                start=(dc == 0),
                stop=(dc == 1),
            )
        mean_sb = small.tile([NL, A], F32)
        nc.vector.tensor_add(mean_sb[:], ps_mean[:], bbt[:])
        nc.sync.dma_start(mean_o[:], mean_sb[:])

        # ---- Jz[(a n), s] = X @ Jproj  (4 col-groups, one per a) ----
        ps_jz = psum.tile([128, S], F32, tag="ps")
        jz_sb = big.tile([128, S], BF16)
        jzT = big.tile([128, 32, 128], BF16)
        for c in range(4):  # 1024-column chunks of s
            jpt = jp_pool.tile([128, 2, A, 1024], BF16)
            for dc in range(2):
                nc.sync.dma_start(
                    jpt[:, dc, :, :],
                    jproj[dc, :, :, c * 1024 : (c + 1) * 1024],
                )
            for dc in range(2):
                for a in range(A):
                    for h in range(2):  # 512-col halves (one PSUM bank)
                        lo = c * 1024 + h * 512
                        nc.tensor.matmul(
                            ps_jz[32 * a : 32 * a + 32, lo : lo + 512],
                            lhsT=xtb[:, dc, :],
                            rhs=jpt[:, dc, a, h * 512 : (h + 1) * 512],
                            start=(dc == 0),
                            stop=(dc == 1),
                            tile_position=(0, 32 * a),
                        )
            # evacuate this 1024-col chunk (split ACT / DVE)
            lo = c * 1024
            nc.scalar.activation(jz_sb[:, lo : lo + 512], ps_jz[:, lo : lo + 512], Copy)
            nc.vector.tensor_copy(
                jz_sb[:, lo + 512 : lo + 1024], ps_jz[:, lo + 512 : lo + 1024]
            )
            for sg in range(8 * c, 8 * c + 8):
                nc.sync.dma_start_transpose(
                    out=jzT[:, sg, :], in_=jz_sb[:, sg * 128 : (sg + 1) * 128]
                )

        # ---- Kzz gram: psum[(a n),(b n')] = sum_s JzT[s,(an)]^T JzT[s,(bn')]
        ps_kzz = psum.tile([128, 128], F32, tag="ps")
        for sg in range(32):
            nc.tensor.matmul(
                ps_kzz[:],
                lhsT=jzT[:, sg, :],
                rhs=jzT[:, sg, :],
                start=(sg == 0),
                stop=(sg == 31),
            )
        kzz_sb = small.tile([128, 128], F32)
        nc.vector.tensor_copy(kzz_sb[:], ps_kzz[:])

        # ---- Kzx[(a n),(b m)] = sum_s JzT^T JxT  (fp8 stream) ----
        ps_kzx = psum.tile([128, MB], F32, tag="ps")
        for sg in range(32):
            jxt_t = jx_pool.tile([128, MB], F8)
            nc.sync.dma_start(jxt_t[:], jxt[sg * 128 : (sg + 1) * 128, :])
            for j in range(8):
                nc.tensor.matmul(
                    ps_kzx[:, j * 512 : (j + 1) * 512],
                    lhsT=jzT[:, sg, :],
                    rhs=jxt_t[:, j * 512 : (j + 1) * 512],
                    start=(sg == 0),
                    stop=(sg == 31),
                )
        kzx_sb = big.tile([128, MB], BF16)
        nc.scalar.activation(
            kzx_sb[:, :2048], ps_kzx[:, :2048], Copy, scale=1.0 / JX_SCALE
        )
        nc.vector.tensor_scalar_mul(
            kzx_sb[:, 2048:], ps_kzx[:, 2048:], 1.0 / JX_SCALE
        )
        kzxT = big.tile([128, 32, 128], BF16)
        for k in range(32):
            nc.sync.dma_start_transpose(
                out=kzxT[:, k, :], in_=kzx_sb[:, k * 128 : (k + 1) * 128]
            )

        # ---- tmp[(a n),(c l)] = sum_(bm) Kzx^T inv  (fp8 stream) ----
        ps_tmp = psum.tile([128, MB], F32, tag="ps")
        for k in range(32):
            inv_t = inv_pool.tile([128, MB], F8)
            nc.sync.dma_start(inv_t[:], invr[k * 128 : (k + 1) * 128, :])
            for j in range(8):
                nc.tensor.matmul(
                    ps_tmp[:, j * 512 : (j + 1) * 512],
                    lhsT=kzxT[:, k, :],
                    rhs=inv_t[:, j * 512 : (j + 1) * 512],
                    start=(k == 0),
                    stop=(k == 31),
                )
        tmp_sb = big.tile([128, MB], BF16)
        nc.scalar.activation(
            tmp_sb[:, :2048], ps_tmp[:, :2048], Copy, scale=1.0 / INV_SCALE
        )
        nc.vector.tensor_scalar_mul(
            tmp_sb[:, 2048:], ps_tmp[:, 2048:], 1.0 / INV_SCALE
        )
        tmpT = big.tile([128, 32, 128], BF16)
        for k in range(32):
            nc.sync.dma_start_transpose(
                out=tmpT[:, k, :], in_=tmp_sb[:, k * 128 : (k + 1) * 128]
            )

        # ---- K2 gram: psum[(a n),(d n')] = sum_(cl) tmpT^T kzxT ----
        ps_k2 = psum.tile([128, 128], F32, tag="ps")
        for k in range(32):
            nc.tensor.matmul(
                ps_k2[:],
                lhsT=tmpT[:, k, :],
                rhs=kzxT[:, k, :],
                start=(k == 0),
                stop=(k == 31),
            )

        # ---- KLLA = Kzz - K2, take per-n diagonal blocks ----
        r1 = small.tile([128, 128], F32)
        nc.vector.scalar_tensor_tensor(
            r1[:],
            in0=ps_k2[:],
            scalar=-1.0,
            in1=kzz_sb[:],
            op0=mybir.AluOpType.mult,
            op1=mybir.AluOpType.add,
        )
        r2 = small.tile([128, 128], F32)
        nc.vector.tensor_mul(r2[:], r1[:], msk[:])
        klla_sb = small.tile([128, A], F32)
        nc.vector.reduce_sum(
            klla_sb[:], r2[:].rearrange("p (d n) -> p d n", d=A), axis=mybir.AxisListType.X
        )
        nc.sync.dma_start(klla_o[:].rearrange("n a d -> a n d"), klla_sb[:])

    nc.finalize()
    return nc


def _get_program():
    global _PROG
    if _PROG is None:
        _PROG = _build_program()
    return _PROG


def _prep_shared(X, W_net, b_net, Jproj, Jx, inv, prior_std):
    X = np.asarray(X, np.float32)
    Jproj_eff = np.asarray(Jproj, np.float32) * np.float32(
        np.asarray(prior_std).reshape(-1)[0]
    )
    # jproj[dc, p, a, s] = Jproj_eff[dc*128 + p, a, s]
    jp = np.ascontiguousarray(
        Jproj_eff.reshape(2, 128, A, S).astype(ml_dtypes.bfloat16)
    )
    # jxt[s, b*M + m] = Jx[m, b, s] * JX_SCALE
    jx_t = np.ascontiguousarray(
        (np.asarray(Jx, np.float32).transpose(2, 1, 0) * JX_SCALE)
        .reshape(S, MB)
        .astype(ml_dtypes.float8_e4m3)
    )
    # invr[b*M + m, c*M + l] = inv[b, c, m, l] * INV_SCALE
    inv_r = np.ascontiguousarray(
        (np.asarray(inv, np.float32).transpose(0, 2, 1, 3) * INV_SCALE)
        .reshape(MB, MB)
        .astype(ml_dtypes.float8_e4m3)
    )
    p = np.arange(128)
    mask = (p[:, None] % 32 == p[None, :] % 32).astype(np.float32)
    wn = np.ascontiguousarray(
        np.asarray(W_net, np.float32).reshape(2, 128, A)
    )
    bbv = np.broadcast_to(
        np.asarray(b_net, np.float32).reshape(1, A), (NL, A)
    ).copy()
    return X, jp, jx_t, inv_r, mask, wn, bbv


def kernel(X, W_net, b_net, Jproj, Jx, inv, prior_std):
    nc = _get_program()
    X, jp, jx_t, inv_r, mask, wn, bbv = _prep_shared(
        X, W_net, b_net, Jproj, Jx, inv, prior_std
    )

    in_maps = []
    for i in range(NCORES):
        xs = X[i * NL : (i + 1) * NL]  # (32, 256)
        xt = np.ascontiguousarray(xs.T.reshape(2, 128, NL))
        in_maps.append(
            {
                "xt_bf": xt.astype(ml_dtypes.bfloat16),
                "xt_f32": xt,
                "wnet": wn,
                "bb": bbv,
                "jproj": jp,
                "jxt": jx_t,
                "invr": inv_r,
                "mask": mask,
            }
        )

    res = run_bass_kernel_spmd(nc, in_maps, list(range(NCORES)))
    mean = np.concatenate([res.results[i]["mean_o"] for i in range(NCORES)], axis=0)
    klla = np.concatenate([res.results[i]["klla_o"] for i in range(NCORES)], axis=0)
    return mean.astype(np.float32), klla.astype(np.float32)


# revision 10
# speedup vs baseline: 1.5049x; 1.5049x over previous
"""Trainium2 Bass kernel for the GP linearized-Laplace (GPLLA) problem.

Math (reference):
    mean = X @ W_net + b_net                                   (n, a)
    Jz   = einsum('nd,das->nas', X, Jproj) * prior_std         (n, a, s)
    Kzz  = einsum('nas,nbs->abn', Jz, Jz)
    Kzx  = einsum('nas,mbs->abnm', Jz, Jx)
    tmp  = einsum('abnm,bcml->acnl', Kzx, inv)
    K2   = einsum('acnl,dcnl->adn', tmp, Kzx)
    out  = (mean, transpose(Kzz - K2, (2,0,1)))                (n,a), (n,a,d)

Sharding: data-parallel over the test-batch axis n (32 points per core on
8 cores); Jx and inv are replicated (streamed from HBM in fp8 with power-
of-two scaling to keep DMA under the per-core HBM roofline).

Per-core layout: rows are the joint index p = a*32 + n_loc (128 = full
partition dim); the big contractions run over s (resp. (b,m)) = 4096 as
32 K-chunks of 128 with fp8 moving operands.  The per-test-point diagonal
of the quadratic form is extracted with a block-diagonal mask multiply +
segmented reduce.  On-chip transposes go through the DMA xbar in four
1024-column chunks per tensor (per-call overhead ~1.2us, so few big calls
beat many small ones).
"""

import sys

for p in ("/opt/trn_rl_repo", "/root/.axon_site/_ro/trn_rl_repo"):
    if p not in sys.path:
        sys.path.insert(0, p)

from contextlib import ExitStack

import ml_dtypes
import numpy as np

import concourse.bass as bass
import concourse.mybir as mybir
import concourse.tile as tile
from concourse import bacc
from concourse.bass_utils import run_bass_kernel_spmd

N_TEST, N_TRAIN, A, D, S = 256, 1024, 4, 256, 4096
NCORES = 8
NL = N_TEST // NCORES          # 32 test points per core
MB = A * N_TRAIN               # 4096, joint (b, m) index
JX_SCALE = 16.0                # fp8 range scaling for Jx (exact power of 2)
INV_SCALE = 1024.0             # fp8 range scaling for inv

F32 = mybir.dt.float32
BF16 = mybir.dt.bfloat16
F8 = mybir.dt.float8e4

_PROG = None


def _build_program():
    nc = bacc.Bacc(
        "TRN2", target_bir_lowering=False, debug=False, num_devices=NCORES
    )

    xt_bf = nc.dram_tensor("xt_bf", [2, 128, NL], BF16, kind="ExternalInput")
    xt_f32 = nc.dram_tensor("xt_f32", [2, 128, NL], F32, kind="ExternalInput")
    wnet = nc.dram_tensor("wnet", [2, 128, A], F32, kind="ExternalInput")
    bb = nc.dram_tensor("bb", [NL, A], F32, kind="ExternalInput")
    jproj = nc.dram_tensor("jproj", [2, 128, A, S], BF16, kind="ExternalInput")
    jxt = nc.dram_tensor("jxt", [S, MB], F8, kind="ExternalInput")
    invr = nc.dram_tensor("invr", [MB, MB], F8, kind="ExternalInput")
    mask = nc.dram_tensor("mask", [128, 128], F32, kind="ExternalInput")

    mean_o = nc.dram_tensor("mean_o", [NL, A], F32, kind="ExternalOutput")
    klla_o = nc.dram_tensor("klla_o", [NL, A, A], F32, kind="ExternalOutput")

    Copy = mybir.ActivationFunctionType.Copy
    Alu = mybir.AluOpType

    with tile.TileContext(nc) as tc, ExitStack() as ctx:
        const = ctx.enter_context(tc.tile_pool(name="const", bufs=1))
        small = ctx.enter_context(tc.tile_pool(name="small", bufs=1))
        big = ctx.enter_context(tc.tile_pool(name="big", bufs=1))
        jp_pool = ctx.enter_context(tc.tile_pool(name="jp", bufs=1))
        jx_pool = ctx.enter_context(tc.tile_pool(name="jx", bufs=6))
        inv_pool = ctx.enter_context(tc.tile_pool(name="inv", bufs=6))
        psum = ctx.enter_context(
            tc.tile_pool(name="psum", bufs=1, space=bass.MemorySpace.PSUM)
        )

        # ---- constants ----
        xtb = const.tile([128, 2, NL], BF16)
        nc.sync.dma_start(xtb[:], xt_bf[:].rearrange("c p n -> p c n"))
        xtf = const.tile([128, 2, NL], F32)
        nc.sync.dma_start(xtf[:], xt_f32[:].rearrange("c p n -> p c n"))
        wn = const.tile([128, 2, A], F32)
        nc.sync.dma_start(wn[:], wnet[:].rearrange("c p n -> p c n"))
        bbt = const.tile([NL, A], F32)
        nc.sync.dma_start(bbt[:], bb[:])
        msk = const.tile([128, 128], F32)
        nc.sync.dma_start(msk[:], mask[:])

        # ---- mean = X @ W_net + b ----
        ps_mean = psum.tile([NL, A], F32, tag="ps")
        for dc in range(2):
            nc.tensor.matmul(
                ps_mean[:],
                lhsT=xtf[:, dc, :],
                rhs=wn[:, dc, :],
                start=(dc == 0),
                stop=(dc == 1),
            )
        mean_sb = small.tile([NL, A], F32)
        nc.vector.tensor_add(mean_sb[:], ps_mean[:], bbt[:])
        nc.sync.dma_start(mean_o[:], mean_sb[:])

        # ---- Jz[(a n), s] = X @ Jproj  (4 col-groups, one per a) ----
        # 8 contiguous 1 MB loads (one per (dchunk, a)) across DMA queues.
        jpa = []
        for dc in range(2):
            for a in range(A):
                t = jp_pool.tile([128, S], BF16, tag=f"jp{dc}{a}")
                nc.sync.dma_start(t[:], jproj[dc, :, a, :])
                jpa.append(t)
        ps_jz = psum.tile([128, S], F32, tag="ps")
        jz_sb = big.tile([128, S], BF16)
        jzT = big.tile([128, 32, 128], BF16)
        for j in range(8):  # 512-col chunks of s
            for dc in range(2):
                for a in range(A):
                    nc.tensor.matmul(
                        ps_jz[32 * a : 32 * a + 32, j * 512 : (j + 1) * 512],
                        lhsT=xtb[:, dc, :],
                        rhs=jpa[dc * 4 + a][:, j * 512 : (j + 1) * 512],
                        start=(dc == 0),
                        stop=(dc == 1),
                        tile_position=(0, 32 * a),
                    )
            # evacuate finished 512-col chunk, alternating ACT/DVE
            lo = j * 512
            if j % 2 == 0:
                nc.scalar.activation(
                    jz_sb[:, lo : lo + 512], ps_jz[:, lo : lo + 512], Copy
                )
            else:
                nc.vector.tensor_copy(
                    jz_sb[:, lo : lo + 512], ps_jz[:, lo : lo + 512]
                )
            if j % 2 == 1:  # 1024-col quarter done -> one big xbar transpose
                q = j // 2
                nc.scalar.dma_start_transpose(
                    out=jzT[:, 8 * q : 8 * q + 8, :],
                    in_=jz_sb[:, q * 1024 : (q + 1) * 1024],
                )

        # ---- Kzz gram: psum[(a n),(b n')] = sum_s JzT[s,(an)]^T JzT[s,(bn')]
        ps_kzz = psum.tile([128, 128], F32, tag="ps")
        for sg in range(32):
            nc.tensor.matmul(
                ps_kzz[:],
                lhsT=jzT[:, sg, :],
                rhs=jzT[:, sg, :],
                start=(sg == 0),
                stop=(sg == 31),
            )
        kzz_sb = small.tile([128, 128], F32)
        nc.vector.tensor_copy(kzz_sb[:], ps_kzz[:])

        # ---- Kzx[(a n),(b m)] = sum_s JzT^T JxT  (fp8 stream) ----
        ps_kzx = psum.tile([128, MB], F32, tag="ps")
        for sg in range(32):
            jxt_t = jx_pool.tile([128, MB], F8)
            nc.sync.dma_start(jxt_t[:], jxt[sg * 128 : (sg + 1) * 128, :])
            for j in range(8):
                nc.tensor.matmul(
                    ps_kzx[:, j * 512 : (j + 1) * 512],
                    lhsT=jzT[:, sg, :],
                    rhs=jxt_t[:, j * 512 : (j + 1) * 512],
                    start=(sg == 0),
                    stop=(sg == 31),
                )
        kzx_sb = big.tile([128, MB], BF16)
        kzxT = big.tile([128, 32, 128], BF16)
        for q in range(4):
            lo = q * 1024
            if q % 2 == 0:
                nc.scalar.activation(
                    kzx_sb[:, lo : lo + 1024],
                    ps_kzx[:, lo : lo + 1024],
                    Copy,
                    scale=1.0 / JX_SCALE,
                )
            else:
                nc.vector.tensor_scalar_mul(
                    kzx_sb[:, lo : lo + 1024],
                    ps_kzx[:, lo : lo + 1024],
                    1.0 / JX_SCALE,
                )
            nc.scalar.dma_start_transpose(
                out=kzxT[:, 8 * q : 8 * q + 8, :],
                in_=kzx_sb[:, lo : lo + 1024],
            )

        # ---- tmp[(a n),(c l)] = sum_(bm) Kzx^T inv  (fp8 stream) ----
        ps_tmp = psum.tile([128, MB], F32, tag="ps")
        for k in range(32):
            inv_t = inv_pool.tile([128, MB], F8)
            nc.sync.dma_start(inv_t[:], invr[k * 128 : (k + 1) * 128, :])
            for j in range(8):
                nc.tensor.matmul(
                    ps_tmp[:, j * 512 : (j + 1) * 512],
                    lhsT=kzxT[:, k, :],
                    rhs=inv_t[:, j * 512 : (j + 1) * 512],
                    start=(k == 0),
                    stop=(k == 31),
                )
        tmp_sb = big.tile([128, MB], BF16)
        tmpT = big.tile([128, 32, 128], BF16)
        for q in range(4):
            lo = q * 1024
            if q % 2 == 0:
                nc.scalar.activation(
                    tmp_sb[:, lo : lo + 1024],
                    ps_tmp[:, lo : lo + 1024],
                    Copy,
                    scale=1.0 / INV_SCALE,
                )
            else:
                nc.vector.tensor_scalar_mul(
                    tmp_sb[:, lo : lo + 1024],
                    ps_tmp[:, lo : lo + 1024],
                    1.0 / INV_SCALE,
                )
            nc.scalar.dma_start_transpose(
                out=tmpT[:, 8 * q : 8 * q + 8, :],
                in_=tmp_sb[:, lo : lo + 1024],
            )

        # ---- K2 gram: psum[(a n),(d n')] = sum_(cl) tmpT^T kzxT ----
        ps_k2 = psum.tile([128, 128], F32, tag="ps")
        for k in range(32):
            nc.tensor.matmul(
                ps_k2[:],
                lhsT=tmpT[:, k, :],
                rhs=kzxT[:, k, :],
                start=(k == 0),
                stop=(k == 31),
            )

        # ---- KLLA = Kzz - K2, take per-n diagonal blocks ----
        r1 = small.tile([128, 128], F32)
        nc.vector.scalar_tensor_tensor(
            r1[:],
            in0=ps_k2[:],
            scalar=-1.0,
            in1=kzz_sb[:],
            op0=Alu.mult,
            op1=Alu.add,
        )
        r2 = small.tile([128, 128], F32)
        nc.vector.tensor_mul(r2[:], r1[:], msk[:])
        klla_sb = small.tile([128, A], F32)
        nc.vector.reduce_sum(
            klla_sb[:],
            r2[:].rearrange("p (d n) -> p d n", d=A),
            axis=mybir.AxisListType.X,
        )
        nc.sync.dma_start(klla_o[:].rearrange("n a d -> a n d"), klla_sb[:])

    nc.finalize()
    return nc


def _get_program():
    global _PROG
    if _PROG is None:
        _PROG = _build_program()
    return _PROG


def _prep_shared(X, W_net, b_net, Jproj, Jx, inv, prior_std):
    X = np.asarray(X, np.float32)
    Jproj_eff = np.asarray(Jproj, np.float32) * np.float32(
        np.asarray(prior_std).reshape(-1)[0]
    )
    # jproj[dc, p, a, s] = Jproj_eff[dc*128 + p, a, s]
    jp = np.ascontiguousarray(
        Jproj_eff.reshape(2, 128, A, S).astype(ml_dtypes.bfloat16)
    )
    # jxt[s, b*M + m] = Jx[m, b, s] * JX_SCALE
    jx_t = np.ascontiguousarray(
        (np.asarray(Jx, np.float32).transpose(2, 1, 0) * JX_SCALE)
        .reshape(S, MB)
        .astype(ml_dtypes.float8_e4m3)
    )
    # invr[b*M + m, c*M + l] = inv[b, c, m, l] * INV_SCALE
    inv_r = np.ascontiguousarray(
        (np.asarray(inv, np.float32).transpose(0, 2, 1, 3) * INV_SCALE)
        .reshape(MB, MB)
        .astype(ml_dtypes.float8_e4m3)
    )
    p = np.arange(128)
    mask = (p[:, None] % 32 == p[None, :] % 32).astype(np.float32)
    wn = np.ascontiguousarray(np.asarray(W_net, np.float32).reshape(2, 128, A))
    bbv = np.broadcast_to(
        np.asarray(b_net, np.float32).reshape(1, A), (NL, A)
    ).copy()
    return X, jp, jx_t, inv_r, mask, wn, bbv


def _make_in_maps(X, jp, jx_t, inv_r, mask, wn, bbv):
    in_maps = []
    for i in range(NCORES):
        xs = np.ascontiguousarray(X[i * NL : (i + 1) * NL])  # (32, 256)
        xt = np.ascontiguousarray(xs.T.reshape(2, 128, NL))
        in_maps.append(
            {
                "xt_bf": xt.astype(ml_dtypes.bfloat16),
                "xt_f32": xt,
                "wnet": wn,
                "bb": bbv,
                "jproj": jp,
                "jxt": jx_t,
                "invr": inv_r,
                "mask": mask,
            }
        )
    return in_maps


def kernel(X, W_net, b_net, Jproj, Jx, inv, prior_std):
    nc = _get_program()
    prep = _prep_shared(X, W_net, b_net, Jproj, Jx, inv, prior_std)
    in_maps = _make_in_maps(*prep)
    res = run_bass_kernel_spmd(nc, in_maps, list(range(NCORES)))
    mean = np.concatenate([res.results[i]["mean_o"] for i in range(NCORES)], axis=0)
    klla = np.concatenate([res.results[i]["klla_o"] for i in range(NCORES)], axis=0)
    return mean.astype(np.float32), klla.astype(np.float32)


# revision 11
# speedup vs baseline: 1.5498x; 1.0298x over previous
"""Trainium2 Bass kernel for the GP linearized-Laplace (GPLLA) problem.

Math (reference):
    mean = X @ W_net + b_net                                   (n, a)
    Jz   = einsum('nd,das->nas', X, Jproj) * prior_std         (n, a, s)
    Kzz  = einsum('nas,nbs->abn', Jz, Jz)
    Kzx  = einsum('nas,mbs->abnm', Jz, Jx)
    tmp  = einsum('abnm,bcml->acnl', Kzx, inv)
    K2   = einsum('acnl,dcnl->adn', tmp, Kzx)
    out  = (mean, transpose(Kzz - K2, (2,0,1)))                (n,a), (n,a,d)

Sharding: data-parallel over the test-batch axis n (32 points per core on
8 cores); Jx and inv are replicated (streamed from HBM in fp8 with power-
of-two scaling to keep DMA under the per-core HBM roofline).

Per-core layout: rows are the joint index p = a*32 + n_loc (128 = full
partition dim); the big contractions run over s (resp. (b,m)) = 4096 as
32 K-chunks of 128 with fp8 moving operands.  The per-test-point diagonal
of the quadratic form is extracted with a block-diagonal mask multiply +
segmented reduce.  On-chip transposes go through the DMA xbar in four
1024-column chunks per tensor (per-call overhead ~1.2us, so few big calls
beat many small ones).
"""

import sys

for p in ("/opt/trn_rl_repo", "/root/.axon_site/_ro/trn_rl_repo"):
    if p not in sys.path:
        sys.path.insert(0, p)

from contextlib import ExitStack

import ml_dtypes
import numpy as np

import concourse.bass as bass
import concourse.mybir as mybir
import concourse.tile as tile
from concourse import bacc
from concourse.bass_utils import run_bass_kernel_spmd

N_TEST, N_TRAIN, A, D, S = 256, 1024, 4, 256, 4096
NCORES = 8
NL = N_TEST // NCORES          # 32 test points per core
MB = A * N_TRAIN               # 4096, joint (b, m) index
JX_SCALE = 16.0                # fp8 range scaling for Jx (exact power of 2)
INV_SCALE = 1024.0             # fp8 range scaling for inv

F32 = mybir.dt.float32
BF16 = mybir.dt.bfloat16
F8 = mybir.dt.float8e4

_PROG = None


def _build_program():
    nc = bacc.Bacc(
        "TRN2", target_bir_lowering=False, debug=False, num_devices=NCORES
    )

    xt_bf = nc.dram_tensor("xt_bf", [2, 128, NL], BF16, kind="ExternalInput")
    xt_f32 = nc.dram_tensor("xt_f32", [2, 128, NL], F32, kind="ExternalInput")
    wnet = nc.dram_tensor("wnet", [2, 128, A], F32, kind="ExternalInput")
    bb = nc.dram_tensor("bb", [NL, A], F32, kind="ExternalInput")
    jproj = nc.dram_tensor("jproj", [2, 128, A, S], BF16, kind="ExternalInput")
    jxt = nc.dram_tensor("jxt", [S, MB], F8, kind="ExternalInput")
    invr = nc.dram_tensor("invr", [MB, MB], F8, kind="ExternalInput")
    mask = nc.dram_tensor("mask", [128, 128], F32, kind="ExternalInput")

    mean_o = nc.dram_tensor("mean_o", [NL, A], F32, kind="ExternalOutput")
    klla_o = nc.dram_tensor("klla_o", [NL, A, A], F32, kind="ExternalOutput")

    Copy = mybir.ActivationFunctionType.Copy
    Alu = mybir.AluOpType

    with tile.TileContext(nc) as tc, ExitStack() as ctx:
        const = ctx.enter_context(tc.tile_pool(name="const", bufs=1))
        small = ctx.enter_context(tc.tile_pool(name="small", bufs=1))
        big = ctx.enter_context(tc.tile_pool(name="big", bufs=1))
        jp_pool = ctx.enter_context(tc.tile_pool(name="jp", bufs=1))
        jx_pool = ctx.enter_context(tc.tile_pool(name="jx", bufs=6))
        inv_pool = ctx.enter_context(tc.tile_pool(name="inv", bufs=6))
        psum = ctx.enter_context(
            tc.tile_pool(name="psum", bufs=1, space=bass.MemorySpace.PSUM)
        )

        # ---- constants ----
        xtb = const.tile([128, 2, NL], BF16)
        nc.sync.dma_start(xtb[:], xt_bf[:].rearrange("c p n -> p c n"))
        xtf = const.tile([128, 2, NL], F32)
        nc.sync.dma_start(xtf[:], xt_f32[:].rearrange("c p n -> p c n"))
        wn = const.tile([128, 2, A], F32)
        nc.sync.dma_start(wn[:], wnet[:].rearrange("c p n -> p c n"))
        bbt = const.tile([NL, A], F32)
        nc.sync.dma_start(bbt[:], bb[:])
        msk = const.tile([128, 128], F32)
        nc.sync.dma_start(msk[:], mask[:])

        # ---- mean = X @ W_net + b ----
        ps_mean = psum.tile([NL, A], F32, tag="ps")
        for dc in range(2):
            nc.tensor.matmul(
                ps_mean[:],
                lhsT=xtf[:, dc, :],
                rhs=wn[:, dc, :],
                start=(dc == 0),
                stop=(dc == 1),
            )
        mean_sb = small.tile([NL, A], F32)
        nc.vector.tensor_add(mean_sb[:], ps_mean[:], bbt[:])
        nc.sync.dma_start(mean_o[:], mean_sb[:])

        # ---- Jz[(a n), s] = X @ Jproj  (4 col-groups, one per a) ----
        # 8 contiguous 1 MB loads (one per (dchunk, a)) across DMA queues.
        jpa = []
        for dc in range(2):
            for a in range(A):
                t = jp_pool.tile([128, S], BF16, tag=f"jp{dc}{a}")
                nc.gpsimd.dma_start(t[:], jproj[dc, :, a, :])
                jpa.append(t)
        ps_jz = psum.tile([128, S], F32, tag="ps")
        jz_sb = big.tile([128, S], BF16)
        jzT = big.tile([128, 32, 128], BF16)
        for j in range(8):  # 512-col chunks of s
            for dc in range(2):
                for a in range(A):
                    nc.tensor.matmul(
                        ps_jz[32 * a : 32 * a + 32, j * 512 : (j + 1) * 512],
                        lhsT=xtb[:, dc, :],
                        rhs=jpa[dc * 4 + a][:, j * 512 : (j + 1) * 512],
                        start=(dc == 0),
                        stop=(dc == 1),
                        tile_position=(0, 32 * a),
                    )
            # evacuate finished 512-col chunk, alternating ACT/DVE
            lo = j * 512
            if j % 2 == 0:
                nc.scalar.activation(
                    jz_sb[:, lo : lo + 512], ps_jz[:, lo : lo + 512], Copy
                )
            else:
                nc.vector.tensor_copy(
                    jz_sb[:, lo : lo + 512], ps_jz[:, lo : lo + 512]
                )
            if j % 2 == 1:  # 1024-col quarter done -> one big xbar transpose
                q = j // 2
                nc.scalar.dma_start_transpose(
                    out=jzT[:, 8 * q : 8 * q + 8, :],
                    in_=jz_sb[:, q * 1024 : (q + 1) * 1024],
                )

        # ---- Kzz gram: psum[(a n),(b n')] = sum_s JzT[s,(an)]^T JzT[s,(bn')]
        ps_kzz = psum.tile([128, 128], F32, tag="ps")
        for sg in range(32):
            nc.tensor.matmul(
                ps_kzz[:],
                lhsT=jzT[:, sg, :],
                rhs=jzT[:, sg, :],
                start=(sg == 0),
                stop=(sg == 31),
            )
        kzz_sb = small.tile([128, 128], F32)
        nc.vector.tensor_copy(kzz_sb[:], ps_kzz[:])

        # ---- Kzx[(a n),(b m)] = sum_s JzT^T JxT  (fp8 stream) ----
        ps_kzx = psum.tile([128, MB], F32, tag="ps")
        for sg in range(32):
            jxt_t = jx_pool.tile([128, MB], F8)
            nc.gpsimd.dma_start(jxt_t[:], jxt[sg * 128 : (sg + 1) * 128, :])
            for j in range(8):
                nc.tensor.matmul(
                    ps_kzx[:, j * 512 : (j + 1) * 512],
                    lhsT=jzT[:, sg, :],
                    rhs=jxt_t[:, j * 512 : (j + 1) * 512],
                    start=(sg == 0),
                    stop=(sg == 31),
                )
        kzx_sb = big.tile([128, MB], BF16)
        kzxT = big.tile([128, 32, 128], BF16)
        for q in range(4):
            lo = q * 1024
            if q % 2 == 0:
                nc.scalar.activation(
                    kzx_sb[:, lo : lo + 1024],
                    ps_kzx[:, lo : lo + 1024],
                    Copy,
                    scale=1.0 / JX_SCALE,
                )
            else:
                nc.vector.tensor_scalar_mul(
                    kzx_sb[:, lo : lo + 1024],
                    ps_kzx[:, lo : lo + 1024],
                    1.0 / JX_SCALE,
                )
            nc.scalar.dma_start_transpose(
                out=kzxT[:, 8 * q : 8 * q + 8, :],
                in_=kzx_sb[:, lo : lo + 1024],
            )

        # ---- tmp[(a n),(c l)] = sum_(bm) Kzx^T inv  (fp8 stream) ----
        ps_tmp = psum.tile([128, MB], F32, tag="ps")
        for k in range(32):
            inv_t = inv_pool.tile([128, MB], F8)
            nc.gpsimd.dma_start(inv_t[:], invr[k * 128 : (k + 1) * 128, :])
            for j in range(8):
                nc.tensor.matmul(
                    ps_tmp[:, j * 512 : (j + 1) * 512],
                    lhsT=kzxT[:, k, :],
                    rhs=inv_t[:, j * 512 : (j + 1) * 512],
                    start=(k == 0),
                    stop=(k == 31),
                )
        tmp_sb = big.tile([128, MB], BF16)
        tmpT = big.tile([128, 32, 128], BF16)
        for q in range(4):
            lo = q * 1024
            if q % 2 == 0:
                nc.scalar.activation(
                    tmp_sb[:, lo : lo + 1024],
                    ps_tmp[:, lo : lo + 1024],
                    Copy,
                    scale=1.0 / INV_SCALE,
                )
            else:
                nc.vector.tensor_scalar_mul(
                    tmp_sb[:, lo : lo + 1024],
                    ps_tmp[:, lo : lo + 1024],
                    1.0 / INV_SCALE,
                )
            nc.scalar.dma_start_transpose(
                out=tmpT[:, 8 * q : 8 * q + 8, :],
                in_=tmp_sb[:, lo : lo + 1024],
            )

        # ---- K2 gram: psum[(a n),(d n')] = sum_(cl) tmpT^T kzxT ----
        ps_k2 = psum.tile([128, 128], F32, tag="ps")
        for k in range(32):
            nc.tensor.matmul(
                ps_k2[:],
                lhsT=tmpT[:, k, :],
                rhs=kzxT[:, k, :],
                start=(k == 0),
                stop=(k == 31),
            )

        # ---- KLLA = Kzz - K2, take per-n diagonal blocks ----
        r1 = small.tile([128, 128], F32)
        nc.vector.scalar_tensor_tensor(
            r1[:],
            in0=ps_k2[:],
            scalar=-1.0,
            in1=kzz_sb[:],
            op0=Alu.mult,
            op1=Alu.add,
        )
        r2 = small.tile([128, 128], F32)
        nc.vector.tensor_mul(r2[:], r1[:], msk[:])
        klla_sb = small.tile([128, A], F32)
        nc.vector.reduce_sum(
            klla_sb[:],
            r2[:].rearrange("p (d n) -> p d n", d=A),
            axis=mybir.AxisListType.X,
        )
        nc.sync.dma_start(klla_o[:].rearrange("n a d -> a n d"), klla_sb[:])

    nc.finalize()
    return nc


def _get_program():
    global _PROG
    if _PROG is None:
        _PROG = _build_program()
    return _PROG


def _prep_shared(X, W_net, b_net, Jproj, Jx, inv, prior_std):
    X = np.asarray(X, np.float32)
    Jproj_eff = np.asarray(Jproj, np.float32) * np.float32(
        np.asarray(prior_std).reshape(-1)[0]
    )
    # jproj[dc, p, a, s] = Jproj_eff[dc*128 + p, a, s]
    jp = np.ascontiguousarray(
        Jproj_eff.reshape(2, 128, A, S).astype(ml_dtypes.bfloat16)
    )
    # jxt[s, b*M + m] = Jx[m, b, s] * JX_SCALE
    jx_t = np.ascontiguousarray(
        (np.asarray(Jx, np.float32).transpose(2, 1, 0) * JX_SCALE)
        .reshape(S, MB)
        .astype(ml_dtypes.float8_e4m3)
    )
    # invr[b*M + m, c*M + l] = inv[b, c, m, l] * INV_SCALE
    inv_r = np.ascontiguousarray(
        (np.asarray(inv, np.float32).transpose(0, 2, 1, 3) * INV_SCALE)
        .reshape(MB, MB)
        .astype(ml_dtypes.float8_e4m3)
    )
    p = np.arange(128)
    mask = (p[:, None] % 32 == p[None, :] % 32).astype(np.float32)
    wn = np.ascontiguousarray(np.asarray(W_net, np.float32).reshape(2, 128, A))
    bbv = np.broadcast_to(
        np.asarray(b_net, np.float32).reshape(1, A), (NL, A)
    ).copy()
    return X, jp, jx_t, inv_r, mask, wn, bbv


def _make_in_maps(X, jp, jx_t, inv_r, mask, wn, bbv):
    in_maps = []
    for i in range(NCORES):
        xs = np.ascontiguousarray(X[i * NL : (i + 1) * NL])  # (32, 256)
        xt = np.ascontiguousarray(xs.T.reshape(2, 128, NL))
        in_maps.append(
            {
                "xt_bf": xt.astype(ml_dtypes.bfloat16),
                "xt_f32": xt,
                "wnet": wn,
                "bb": bbv,
                "jproj": jp,
                "jxt": jx_t,
                "invr": inv_r,
                "mask": mask,
            }
        )
    return in_maps


def kernel(X, W_net, b_net, Jproj, Jx, inv, prior_std):
    nc = _get_program()
    prep = _prep_shared(X, W_net, b_net, Jproj, Jx, inv, prior_std)
    in_maps = _make_in_maps(*prep)
    res = run_bass_kernel_spmd(nc, in_maps, list(range(NCORES)))
    mean = np.concatenate([res.results[i]["mean_o"] for i in range(NCORES)], axis=0)
    klla = np.concatenate([res.results[i]["klla_o"] for i in range(NCORES)], axis=0)
    return mean.astype(np.float32), klla.astype(np.float32)
